# revision 2
# baseline (speedup 1.0000x reference)
"""Causal (cumulative) LayerNorm Trainium2 Bass kernel.

Full-input contract: kernel(inputs, gamma, beta) takes the full
(B=8, K=16000, H=256) f32 tensor, shards batch across 8 NeuronCores
(one sample per core), and returns the full (8, 16000, 256) output.

Per-core algorithm (x is (K, H)):
  rowsum[k]  = sum_h x[k, h]
  rowsumsq[k] = sum_h x[k, h]^2
  csum = cumsum(rowsum); cpow = cumsum(rowsumsq)
  mean[k] = csum[k] / (H*(k+1));  msq[k] = cpow[k] / (H*(k+1))
  var[k] = msq[k] - mean[k]^2
  out[k, h] = gamma[h] * (x[k, h] - mean[k]) / sqrt(var[k] + EPS) + beta[h]

Layout: K = 16000 = 125 tiles x 128 rows. X lives in SBUF tile-major as
(128 part, 125 tile, 256 h), row k = t*128 + p at [p, t, :]. Row stats
are accumulated tile-major (128, 125), transposed (exact, PE transpose)
to chunk-major (125, 128) where partition t holds 128 consecutive rows,
scanned along the free axis (vector engine fp32 scan), carry-corrected
across partitions with one more transpose pair, and the resulting
per-row scale/bias transposed back to tile-major for the output pass.
"""

import numpy as np

import concourse.bass as bass
import concourse.bacc as bacc
import concourse.tile as tile
from concourse import mybir
from concourse.bass_utils import run_bass_kernel_spmd

EPS = 1e-8
B, K, H = 8, 16000, 256
P = 128                 # SBUF partitions
NT = K // P             # 125 row-tiles per sample
G = 5                   # tiles per DMA group
NG = NT // G            # 25 DMA groups
F32 = mybir.dt.float32
ALU = mybir.AluOpType
ACT = mybir.ActivationFunctionType


def _build(use_beta: bool):
    nc = bacc.Bacc("TRN2", target_bir_lowering=False, debug=False)

    x = nc.declare_dram_parameter("x", [K, H], F32, isOutput=False)
    gamma_b = nc.declare_dram_parameter("gamma_b", [P, H], F32, isOutput=False)
    beta_b = (
        nc.declare_dram_parameter("beta_b", [P, H], F32, isOutput=False)
        if use_beta
        else None
    )
    ident = nc.declare_dram_parameter("ident", [P, P], F32, isOutput=False)
    invc = nc.declare_dram_parameter("invc", [NT, P], F32, isOutput=False)
    y = nc.declare_dram_parameter("y", [K, H], F32, isOutput=True)

    xr = x.rearrange("(t p) h -> p t h", p=P)   # [128, 125, 256]
    yr = y.rearrange("(t p) h -> p t h", p=P)

    with tile.TileContext(nc) as tc:
        with (
            tc.tile_pool(name="singles", bufs=1) as singles,
            tc.tile_pool(name="xpool", bufs=NG) as xpool,
            tc.tile_pool(name="opool", bufs=4) as opool,
            tc.tile_pool(name="sqpool", bufs=3) as sqpool,
            tc.tile_pool(name="psum", bufs=1, space="PSUM") as psum,
        ):
            sb_gamma = singles.tile([P, H], F32)
            nc.sync.dma_start(out=sb_gamma[:], in_=gamma_b[:])
            if use_beta:
                sb_beta = singles.tile([P, H], F32)
                nc.sync.dma_start(out=sb_beta[:], in_=beta_b[:])
            sb_ident = singles.tile([P, P], F32)
            nc.sync.dma_start(out=sb_ident[:], in_=ident[:])
            sb_invc = singles.tile([NT, P], F32)
            nc.sync.dma_start(out=sb_invc[:], in_=invc[:])

            s_sum = singles.tile([P, NT], F32)   # tile-major row sums
            s_pow = singles.tile([P, NT], F32)   # tile-major row sum-squares

            # Phase 1: stream X in, accumulate per-row sums.
            xtiles = []
            for g in range(NG):
                xt = xpool.tile([P, G, H], F32)
                nc.sync.dma_start(out=xt[:], in_=xr[:, g * G:(g + 1) * G, :])
                xtiles.append(xt)
                for j in range(G):
                    t = g * G + j
                    nc.vector.reduce_sum(
                        out=s_sum[:, t:t + 1],
                        in_=xt[:, j, :],
                        axis=mybir.AxisListType.X,
                    )
                    sq = sqpool.tile([P, H], F32)
                    nc.scalar.activation(
                        out=sq[:],
                        in_=xt[:, j, :],
                        func=ACT.Square,
                        accum_out=s_pow[:, t:t + 1],
                    )

            # Phase 2: global prefix sums + per-row scale/bias.
            # Tile-major (128, 125) -> chunk-major (125, 128): partition t
            # holds rows [t*128, (t+1)*128) in order.
            ps_s = psum.tile([NT, P], F32)
            nc.tensor.transpose(ps_s[:], s_sum[:], sb_ident[:])
            ps_p = psum.tile([NT, P], F32)
            nc.tensor.transpose(ps_p[:], s_pow[:], sb_ident[:])

            scan_s = singles.tile([NT, P], F32)
            nc.vector.tensor_tensor_scan(
                out=scan_s[:], data0=ps_s[:], data1=sb_invc[:],
                initial=0.0, op0=ALU.add, op1=ALU.bypass,
            )
            scan_p = singles.tile([NT, P], F32)
            nc.vector.tensor_tensor_scan(
                out=scan_p[:], data0=ps_p[:], data1=sb_invc[:],
                initial=0.0, op0=ALU.add, op1=ALU.bypass,
            )

            # Per-chunk totals, exclusive-scanned across chunks.
            tot = singles.tile([NT, 2], F32)
            nc.vector.tensor_copy(out=tot[:, 0:1], in_=scan_s[:, P - 1:P])
            nc.vector.tensor_copy(out=tot[:, 1:2], in_=scan_p[:, P - 1:P])
            pt = psum.tile([2, NT], F32)
            nc.tensor.transpose(pt[:], tot[:], sb_ident[0:NT, 0:NT])
            excl = singles.tile([2, NT], F32)
            nc.vector.memset(excl[:, 0:1], 0.0)
            nc.vector.tensor_tensor_scan(
                out=excl[:, 1:NT], data0=pt[:, 0:NT - 1],
                data1=sb_invc[0:2, 0:NT - 1],
                initial=0.0, op0=ALU.add, op1=ALU.bypass,
            )
            ps_o = psum.tile([NT, 2], F32)
            nc.tensor.transpose(ps_o[:], excl[:], sb_ident[0:2, 0:2])

            # mean = (scan + carry) * invc ; msq likewise (invc = 1/(H*count))
            mean_c = singles.tile([NT, P], F32)
            nc.vector.scalar_tensor_tensor(
                out=mean_c[:], in0=scan_s[:], scalar=ps_o[:, 0:1],
                in1=sb_invc[:], op0=ALU.add, op1=ALU.mult,
            )
            msq_c = singles.tile([NT, P], F32)
            nc.vector.scalar_tensor_tensor(
                out=msq_c[:], in0=scan_p[:], scalar=ps_o[:, 1:2],
                in1=sb_invc[:], op0=ALU.add, op1=ALU.mult,
            )
            # var = msq - mean^2
            var_c = singles.tile([NT, P], F32)
            nc.vector.tensor_mul(out=var_c[:], in0=mean_c[:], in1=mean_c[:])
            nc.vector.tensor_sub(out=var_c[:], in0=msq_c[:], in1=var_c[:])
            # inv = 1/sqrt(var + eps)
            sb_eps = singles.tile([NT, 1], F32)
            nc.vector.memset(sb_eps[:], EPS)
            sd_c = singles.tile([NT, P], F32)
            nc.scalar.activation(
                out=sd_c[:], in_=var_c[:], func=ACT.Sqrt, bias=sb_eps[:],
            )
            inv_c = singles.tile([NT, P], F32)
            nc.vector.reciprocal(out=inv_c[:], in_=sd_c[:])
            # nmi = -mean * inv  (bias for the output affine pass)
            nmi_c = singles.tile([NT, P], F32)
            nc.vector.scalar_tensor_tensor(
                out=nmi_c[:], in0=mean_c[:], scalar=-1.0, in1=inv_c[:],
                op0=ALU.mult, op1=ALU.mult,
            )

            # Back to tile-major (128, 125) for per-partition scale/bias.
            ps_inv = psum.tile([P, NT], F32)
            nc.tensor.transpose(ps_inv[:], inv_c[:], sb_ident[0:NT, 0:NT])
            ps_nmi = psum.tile([P, NT], F32)
            nc.tensor.transpose(ps_nmi[:], nmi_c[:], sb_ident[0:NT, 0:NT])
            inv_t = singles.tile([P, NT], F32)
            nc.vector.tensor_copy(out=inv_t[:], in_=ps_inv[:])
            nmi_t = singles.tile([P, NT], F32)
            nc.vector.tensor_copy(out=nmi_t[:], in_=ps_nmi[:])

            # Phase 3: normalize + gamma (+ beta), stream out.
            for g in range(NG):
                ob = opool.tile([P, G, H], F32)
                xt = xtiles[g]
                for j in range(G):
                    t = g * G + j
                    nc.scalar.activation(
                        out=ob[:, j, :], in_=xt[:, j, :], func=ACT.Identity,
                        bias=nmi_t[:, t:t + 1], scale=inv_t[:, t:t + 1],
                    )
                    nc.vector.tensor_mul(
                        out=ob[:, j, :], in0=ob[:, j, :], in1=sb_gamma[:],
                    )
                    if use_beta:
                        nc.vector.tensor_add(
                            out=ob[:, j, :], in0=ob[:, j, :], in1=sb_beta[:],
                        )
                nc.sync.dma_start(out=yr[:, g * G:(g + 1) * G, :], in_=ob[:])

    nc.compile()
    return nc


_CACHE = {}


def _get(use_beta: bool):
    if use_beta not in _CACHE:
        _CACHE[use_beta] = _build(use_beta)
    return _CACHE[use_beta]


def kernel(inputs: np.ndarray, gamma: np.ndarray, beta: np.ndarray) -> np.ndarray:
    inputs = np.ascontiguousarray(inputs, dtype=np.float32)
    gamma = np.asarray(gamma, dtype=np.float32).reshape(1, H)
    beta = np.asarray(beta, dtype=np.float32).reshape(1, H)
    use_beta = bool(np.any(beta))

    nc = _get(use_beta)

    gamma_b = np.ascontiguousarray(np.broadcast_to(gamma, (P, H)))
    ident = np.eye(P, dtype=np.float32)
    counts = 1.0 / (H * (np.arange(K, dtype=np.float64) + 1.0))
    invc = counts.reshape(NT, P).astype(np.float32)   # [t, p] -> row t*128+p

    in_maps = []
    for b in range(B):
        m = {
            "x": np.ascontiguousarray(inputs[b]),
            "gamma_b": gamma_b,
            "ident": ident,
            "invc": invc,
        }
        if use_beta:
            m["beta_b"] = np.ascontiguousarray(np.broadcast_to(beta, (P, H)))
        in_maps.append(m)

    res = run_bass_kernel_spmd(nc, in_maps, list(range(B)))
    out = np.stack([res.results[b]["y"] for b in range(B)], axis=0)
    return out


# revision 4
# speedup vs baseline: 1.1499x; 1.1499x over previous
"""Causal (cumulative) LayerNorm Trainium2 Bass kernel.

Full-input contract: kernel(inputs, gamma, beta) takes the full
(B=8, K=16000, H=256) f32 tensor, shards batch across 8 NeuronCores
(one sample per core), and returns the full (8, 16000, 256) output.

Per-core algorithm (x is (K, H)):
  rowsum[k]   = sum_h x[k, h]
  rowsumsq[k] = sum_h x[k, h]^2
  csum = cumsum(rowsum); cpow = cumsum(rowsumsq)
  mean[k] = csum[k] / (H*(k+1));  msq[k] = cpow[k] / (H*(k+1))
  var[k] = msq[k] - mean[k]^2
  out[k, h] = gamma[h] * (x[k, h] - mean[k]) / sqrt(var[k] + EPS) + beta[h]

Layout: K = 16000 = 125 tiles x 128 rows. X lives in SBUF tile-major as
(128 part, 125 tile, 256 h), row k = t*128 + p at [p, t, :]. Per-row
sums come from one bn_stats per tile (mean/M2 of even/odd elements,
merged with six full-width vector ops). Tile-major stats (128, 125) are
transposed (exact PE transpose) to chunk-major (125, 128) where
partition t holds 128 consecutive rows, scanned along the free axis
(fp32 vector scan), carry-corrected across partitions via one more
transpose pair, and the per-row scale/bias transposed back to
tile-major for the output pass. Output pass: per-tile affine on the
scalar engine, batched gamma multiply split between gpsimd and vector.
"""

import numpy as np

import concourse.bass as bass
import concourse.bacc as bacc
import concourse.tile as tile
from concourse import mybir
from concourse.bass_utils import run_bass_kernel_spmd

EPS = 1e-8
B, K, H = 8, 16000, 256
P = 128                 # SBUF partitions
NT = K // P             # 125 row-tiles per sample
G = 5                   # tiles per DMA group
NG = NT // G            # 25 DMA groups
F32 = mybir.dt.float32
ALU = mybir.AluOpType
ACTF = mybir.ActivationFunctionType

# fraction of gamma-multiply groups routed to gpsimd: group g goes to
# gpsimd unless g % GP_MOD == 0
GP_MOD = 3


def _build(use_beta: bool):
    nc = bacc.Bacc("TRN2", target_bir_lowering=False, debug=False)

    x = nc.declare_dram_parameter("x", [K, H], F32, isOutput=False)
    gamma_b = nc.declare_dram_parameter("gamma_b", [P, H], F32, isOutput=False)
    beta_b = (
        nc.declare_dram_parameter("beta_b", [P, H], F32, isOutput=False)
        if use_beta
        else None
    )
    ident = nc.declare_dram_parameter("ident", [P, P], F32, isOutput=False)
    invc_m = nc.declare_dram_parameter("invc_m", [NT, P], F32, isOutput=False)
    invc_p = nc.declare_dram_parameter("invc_p", [NT, P], F32, isOutput=False)
    y = nc.declare_dram_parameter("y", [K, H], F32, isOutput=True)

    xr = x.rearrange("(t p) h -> p t h", p=P)   # [128, 125, 256]
    yr = y.rearrange("(t p) h -> p t h", p=P)

    with tile.TileContext(nc) as tc:
        with (
            tc.tile_pool(name="singles", bufs=1) as singles,
            tc.tile_pool(name="xpool", bufs=NG) as xpool,
            tc.tile_pool(name="opool", bufs=6) as opool,
            tc.tile_pool(name="psum", bufs=1, space="PSUM") as psum,
        ):
            sb_gamma = singles.tile([P, H], F32)
            nc.sync.dma_start(out=sb_gamma[:], in_=gamma_b[:])
            if use_beta:
                sb_beta = singles.tile([P, H], F32)
                nc.sync.dma_start(out=sb_beta[:], in_=beta_b[:])
            sb_ident = singles.tile([P, P], F32)
            nc.sync.dma_start(out=sb_ident[:], in_=ident[:])
            sb_invm = singles.tile([NT, P], F32)
            nc.sync.dma_start(out=sb_invm[:], in_=invc_m[:])
            sb_invp = singles.tile([NT, P], F32)
            nc.sync.dma_start(out=sb_invp[:], in_=invc_p[:])

            bn = singles.tile([P, NT, 6], F32)  # per-row bn_stats

            # Phase 1: stream X in, per-row bn stats.
            xtiles = []
            for g in range(NG):
                xt = xpool.tile([P, G, H], F32)
                nc.sync.dma_start(out=xt[:], in_=xr[:, g * G:(g + 1) * G, :])
                xtiles.append(xt)
                for j in range(G):
                    t = g * G + j
                    nc.vector.bn_stats(out=bn[:, t, :], in_=xt[:, j, :])

            # Merge even/odd halves: rowsum/128 and rowsumsq, tile-major.
            me = bn[:, :, 1]
            mo = bn[:, :, 4]
            m2e = bn[:, :, 2]
            m2o = bn[:, :, 5]
            s_sum = singles.tile([P, NT], F32)   # rowsum / 128
            nc.vector.tensor_add(out=s_sum[:], in0=me, in1=mo)
            m2 = singles.tile([P, NT], F32)
            nc.vector.tensor_add(out=m2[:], in0=m2e, in1=m2o)
            pe = singles.tile([P, NT], F32)
            nc.vector.tensor_mul(out=pe[:], in0=me, in1=me)
            po = singles.tile([P, NT], F32)
            nc.vector.tensor_mul(out=po[:], in0=mo, in1=mo)
            nc.vector.tensor_add(out=pe[:], in0=pe[:], in1=po[:])
            s_pow = singles.tile([P, NT], F32)   # rowsumsq
            nc.vector.scalar_tensor_tensor(
                out=s_pow[:], in0=pe[:], scalar=128.0, in1=m2[:],
                op0=ALU.mult, op1=ALU.add,
            )

            # Phase 2: global prefix sums + per-row scale/bias.
            ps_s = psum.tile([NT, P], F32)
            nc.tensor.transpose(ps_s[:], s_sum[:], sb_ident[:])
            ps_p = psum.tile([NT, P], F32)
            nc.tensor.transpose(ps_p[:], s_pow[:], sb_ident[:])

            scan_s = singles.tile([NT, P], F32)
            nc.vector.tensor_tensor_scan(
                out=scan_s[:], data0=ps_s[:], data1=sb_invm[:],
                initial=0.0, op0=ALU.add, op1=ALU.bypass,
            )
            scan_p = singles.tile([NT, P], F32)
            nc.vector.tensor_tensor_scan(
                out=scan_p[:], data0=ps_p[:], data1=sb_invm[:],
                initial=0.0, op0=ALU.add, op1=ALU.bypass,
            )

            # Per-chunk totals, exclusive-scanned across chunks.
            tot = singles.tile([NT, 2], F32)
            nc.vector.tensor_copy(out=tot[:, 0:1], in_=scan_s[:, P - 1:P])
            nc.vector.tensor_copy(out=tot[:, 1:2], in_=scan_p[:, P - 1:P])
            pt = psum.tile([2, NT], F32)
            nc.tensor.transpose(pt[:], tot[:], sb_ident[0:NT, 0:NT])
            excl = singles.tile([2, NT], F32)
            nc.vector.memset(excl[:, 0:1], 0.0)
            nc.vector.tensor_tensor_scan(
                out=excl[:, 1:NT], data0=pt[:, 0:NT - 1],
                data1=sb_invm[0:2, 0:NT - 1],
                initial=0.0, op0=ALU.add, op1=ALU.bypass,
            )
            ps_o = psum.tile([NT, 2], F32)
            nc.tensor.transpose(ps_o[:], excl[:], sb_ident[0:2, 0:2])

            # mean = (scan + carry) * invc_m ; msq = (scan + carry) * invc_p
            mean_c = singles.tile([NT, P], F32)
            nc.vector.scalar_tensor_tensor(
                out=mean_c[:], in0=scan_s[:], scalar=ps_o[:, 0:1],
                in1=sb_invm[:], op0=ALU.add, op1=ALU.mult,
            )
            msq_c = singles.tile([NT, P], F32)
            nc.vector.scalar_tensor_tensor(
                out=msq_c[:], in0=scan_p[:], scalar=ps_o[:, 1:2],
                in1=sb_invp[:], op0=ALU.add, op1=ALU.mult,
            )
            # var = msq - mean^2
            var_c = singles.tile([NT, P], F32)
            nc.vector.tensor_mul(out=var_c[:], in0=mean_c[:], in1=mean_c[:])
            nc.vector.tensor_sub(out=var_c[:], in0=msq_c[:], in1=var_c[:])
            # inv = 1/sqrt(var + eps)
            sb_eps = singles.tile([NT, 1], F32)
            nc.vector.memset(sb_eps[:], EPS)
            sd_c = singles.tile([NT, P], F32)
            nc.scalar.activation(
                out=sd_c[:], in_=var_c[:], func=ACTF.Sqrt, bias=sb_eps[:],
            )
            inv_c = singles.tile([NT, P], F32)
            nc.vector.reciprocal(out=inv_c[:], in_=sd_c[:])
            # nmi = -mean * inv  (bias for the output affine pass)
            nmi_c = singles.tile([NT, P], F32)
            nc.vector.scalar_tensor_tensor(
                out=nmi_c[:], in0=mean_c[:], scalar=-1.0, in1=inv_c[:],
                op0=ALU.mult, op1=ALU.mult,
            )

            # Back to tile-major (128, 125) for per-partition scale/bias.
            ps_inv = psum.tile([P, NT], F32)
            nc.tensor.transpose(ps_inv[:], inv_c[:], sb_ident[0:NT, 0:NT])
            ps_nmi = psum.tile([P, NT], F32)
            nc.tensor.transpose(ps_nmi[:], nmi_c[:], sb_ident[0:NT, 0:NT])
            inv_t = singles.tile([P, NT], F32)
            nc.vector.tensor_copy(out=inv_t[:], in_=ps_inv[:])
            nmi_t = singles.tile([P, NT], F32)
            nc.vector.tensor_copy(out=nmi_t[:], in_=ps_nmi[:])

            # Phase 3: normalize + gamma (+ beta), stream out.
            gamma_bc = sb_gamma[:].rearrange("p (o h) -> p o h", o=1).to_broadcast(
                (P, G, H)
            )
            if use_beta:
                beta_bc = sb_beta[:].rearrange("p (o h) -> p o h", o=1).to_broadcast(
                    (P, G, H)
                )
            for g in range(NG):
                ob = opool.tile([P, G, H], F32)
                xt = xtiles[g]
                for j in range(G):
                    t = g * G + j
                    nc.scalar.activation(
                        out=ob[:, j, :], in_=xt[:, j, :], func=ACTF.Identity,
                        bias=nmi_t[:, t:t + 1], scale=inv_t[:, t:t + 1],
                    )
                eng = nc.vector if g % GP_MOD == 0 else nc.gpsimd
                eng.tensor_mul(out=ob[:], in0=ob[:], in1=gamma_bc)
                if use_beta:
                    eng.tensor_add(out=ob[:], in0=ob[:], in1=beta_bc)
                nc.sync.dma_start(out=yr[:, g * G:(g + 1) * G, :], in_=ob[:])

    nc.compile()
    return nc


_CACHE = {}


def _get(use_beta: bool):
    if use_beta not in _CACHE:
        _CACHE[use_beta] = _build(use_beta)
    return _CACHE[use_beta]


def _make_consts():
    gamma_ident = np.eye(P, dtype=np.float32)
    counts = np.arange(K, dtype=np.float64) + 1.0
    invc_m = (1.0 / (2.0 * counts)).reshape(NT, P).astype(np.float32)
    invc_p = (1.0 / (float(H) * counts)).reshape(NT, P).astype(np.float32)
    return gamma_ident, invc_m, invc_p


def _prepare(inputs, gamma, beta):
    inputs = np.ascontiguousarray(inputs, dtype=np.float32)
    gamma = np.asarray(gamma, dtype=np.float32).reshape(1, H)
    beta = np.asarray(beta, dtype=np.float32).reshape(1, H)
    use_beta = bool(np.any(beta))

    gamma_b = np.ascontiguousarray(np.broadcast_to(gamma, (P, H)))
    ident, invc_m, invc_p = _make_consts()

    in_maps = []
    for b in range(B):
        m = {
            "x": np.ascontiguousarray(inputs[b]),
            "gamma_b": gamma_b,
            "ident": ident,
            "invc_m": invc_m,
            "invc_p": invc_p,
        }
        if use_beta:
            m["beta_b"] = np.ascontiguousarray(np.broadcast_to(beta, (P, H)))
        in_maps.append(m)
    return use_beta, in_maps


def kernel(inputs: np.ndarray, gamma: np.ndarray, beta: np.ndarray) -> np.ndarray:
    use_beta, in_maps = _prepare(inputs, gamma, beta)
    nc = _get(use_beta)
    res = run_bass_kernel_spmd(nc, in_maps, list(range(B)))
    out = np.stack([res.results[b]["y"] for b in range(B)], axis=0)
    return out
